# revision 3
# baseline (speedup 1.0000x reference)
"""Trainium2 Bass kernel for nn_AngleFreqEnhance.

out = x + w_out . real(IFFT2(gain ⊙ FFT2(w_in . x)))

The gain mask is data-independent given `weights`; it is computed host-side
(replicating the reference's jax-f32 ops bit-for-bit) and folded with the
ifftshift.  FFTs are DFT matmuls on the TensorEngine with a 4-stage
lhsT=data / lhsT=const alternating chain that absorbs every transpose:

    A1 = X^T F          (lhsT = X chunks,  rhs = [Fr|Fi] concat)
    S  = A1^T F = FXF   (lhsT = A1 chunks, rhs = [Fr|Fi], [-Fi|Fr])
    P  = g ⊙ S          (DVE multiply during PSUM drain)
    A3 = P^T conj(F)    (lhsT = P chunks,  rhs = [Fr|-Fi], [Fi|Fr])
    E  = real(A3^T conj(F)) = A3r^T Fr + A3i^T Fi

Sharding: batch-row slabs.  Core i owns image rows [32i, 32i+32) of every
(b, c) for the pixelwise phases, and owns mid-channels {2i, 2i+1} for the
FFT phase.  Two AllToAlls per batch redistribute xp / x_enh.

x is shipped as bf16 (phase-1 rhs + final residual add) which halves input
DMA traffic and SBUF residency; the FFT chain itself is f32.
"""

import numpy as np
import ml_dtypes

EPS = 1e-8
N_ANGLES = 8
RADIUS_WIDTH = 8
HIGH_FREQ_RATIO = 0.8
OVERLAP_RATIO = 1.5
WEIGHT_RANGE = 0.5

B, C, H, W = 4, 256, 256, 256
CMID = 16
NCORES = 8
HSH = H // NCORES          # 32 rows per core
PX = HSH * W               # 8192 pixels per (b, core)
NG = PX // 512             # 16 groups of 512 px

_CACHE = {}


# ----------------------------------------------------------------- host math
def _gain_masks(weights):
    """[16, H, W] f32 gain in UNSHIFTED frequency coords (ifftshift folded).

    Replicates reference.py's mask math with jax-f32 on CPU so boundary
    pixels (the f32 `% pi` wraparound) match the reference bit-for-bit."""
    import jax
    import jax.numpy as jnp

    with jax.default_device(jax.devices('cpu')[0]):
        cy, cx = H // 2, W // 2
        yy = jnp.arange(H, dtype=jnp.float32)[:, None]
        xx = jnp.arange(W, dtype=jnp.float32)[None, :]
        r = jnp.sqrt((yy - cy) ** 2 + (xx - cx) ** 2)
        theta = (jnp.arctan2(yy - cy, xx - cx) + np.pi) % np.pi
        max_r = max(cy, cx)
        n_radii = int(max_r // RADIUS_WIDTH) + 1
        ridx = jnp.clip(jnp.floor(r / RADIUS_WIDTH).astype(jnp.int32), 0, n_radii - 1)

        delta = np.pi / N_ANGLES
        half_width = OVERLAP_RATIO * delta / 2.0
        centers = (jnp.arange(N_ANGLES, dtype=jnp.float32) * delta + delta / 2.0)[:, None, None]
        dist = jnp.abs(theta[None] - centers)
        aw = jnp.where(dist < half_width, jnp.clip(1.0 - dist / half_width, 0.0, None), 0.0)
        aw = aw / (aw.sum(axis=0, keepdims=True) + EPS)

        w_map = jnp.asarray(weights)[:, :, ridx]          # [16, A, H, W]
        gain = jnp.einsum('ahw,cahw->chw', aw, w_map)
        gain = 1.0 + WEIGHT_RANGE * jnp.tanh(gain)
        hf = r > HIGH_FREQ_RATIO * max_r
        gain = jnp.where(hf[None], gain, 1.0)
        gain = np.asarray(gain)
    return np.fft.ifftshift(gain, axes=(-2, -1)).astype(np.float32)


def _chunk_concat(A, Bm):
    """[256,256] pair -> [128, 1024]: cols c*512+[0:256]=A rows chunk c,
    c*512+[256:512]=B rows chunk c."""
    out = np.zeros((128, 1024), np.float32)
    for c in range(2):
        out[:, c * 512:c * 512 + 256] = A[c * 128:(c + 1) * 128, :]
        out[:, c * 512 + 256:(c + 1) * 512] = Bm[c * 128:(c + 1) * 128, :]
    return out


def _dft_consts():
    N = H
    jj, kk = np.meshgrid(np.arange(N), np.arange(N), indexing='ij')
    F = np.exp(-2j * np.pi * jj * kk / N) / np.sqrt(N)
    Fr = F.real.astype(np.float32)
    Fi = F.imag.astype(np.float32)
    nFi = (-Fi).astype(np.float32)
    return {
        "c0": _chunk_concat(Fr, Fi),
        "c3": _chunk_concat(nFi, Fr),
        "c1": _chunk_concat(Fr, nFi),
        "c2": _chunk_concat(Fi, Fr),
    }


# ------------------------------------------------------------- device graph
def build_graph():
    import concourse.mybir as mybir
    import concourse.tile as tile
    import concourse.bacc as bacc

    f32 = mybir.dt.float32
    bf16 = mybir.dt.bfloat16

    nc = bacc.Bacc("TRN2", target_bir_lowering=False, debug=False, num_devices=NCORES)

    xs = nc.dram_tensor("xs", [B, C, HSH, W], bf16, kind="ExternalInput").ap()
    c0 = nc.dram_tensor("c0", [128, 1024], f32, kind="ExternalInput").ap()
    c1 = nc.dram_tensor("c1", [128, 1024], f32, kind="ExternalInput").ap()
    c2 = nc.dram_tensor("c2", [128, 1024], f32, kind="ExternalInput").ap()
    c3 = nc.dram_tensor("c3", [128, 1024], f32, kind="ExternalInput").ap()
    gm = nc.dram_tensor("gm", [128, 1024], f32, kind="ExternalInput").ap()
    wi = nc.dram_tensor("wi", [128, 32], bf16, kind="ExternalInput").ap()
    wo = nc.dram_tensor("wo", [128, 2048], f32, kind="ExternalInput").ap()
    out = nc.dram_tensor("out", [B, C, HSH, W], f32, kind="ExternalOutput").ap()

    RG = [list(range(NCORES))]

    with tile.TileContext(nc) as tc:
        with (
            tc.tile_pool(name="dram", bufs=1, space="DRAM") as dpool,
            tc.tile_pool(name="const", bufs=1) as cpool,
            tc.tile_pool(name="work", bufs=1) as wk,
            tc.tile_pool(name="ps", bufs=1, space="PSUM") as ps,
        ):
            fwd_in = [dpool.tile([CMID, NG, 512], f32, name=f"fwdin{b}",
                                 tag=f"fwdin{b}") for b in range(B)]
            fwd_out = [dpool.tile([NCORES, 2, HSH, W], f32, name=f"fwdout{b}",
                                  tag=f"fwdout{b}") for b in range(B)]
            bwd_in = [dpool.tile([NCORES, 2, HSH, W], f32, name=f"bwdin{b}",
                                 tag=f"bwdin{b}") for b in range(B)]
            bwd_out = [dpool.tile([CMID, NG, 512], f32, name=f"bwdout{b}",
                                  tag=f"bwdout{b}") for b in range(B)]

            c0_sb = cpool.tile([128, 1024], f32, name="c0sb")
            c1_sb = cpool.tile([128, 1024], f32, name="c1sb")
            c2_sb = cpool.tile([128, 1024], f32, name="c2sb")
            c3_sb = cpool.tile([128, 1024], f32, name="c3sb")
            gm_sb = cpool.tile([128, 1024], f32, name="gmsb")
            wi_sb = cpool.tile([128, 32], bf16, name="wisb")
            wo_sb = cpool.tile([128, 2048], f32, name="wosb")
            nc.sync.dma_start(c0_sb[:, :], c0[:, :])
            nc.sync.dma_start(c1_sb[:, :], c1[:, :])
            nc.sync.dma_start(c2_sb[:, :], c2[:, :])
            nc.sync.dma_start(c3_sb[:, :], c3[:, :])
            nc.sync.dma_start(gm_sb[:, :], gm[:, :])
            nc.sync.dma_start(wi_sb[:, :], wi[:, :])
            nc.sync.dma_start(wo_sb[:, :], wo[:, :])

            for b in range(B):
                # ---- load x[b] slab (bf16), layout [c-in-chunk, chunk*8192+px]
                x_sb = wk.tile([128, 2 * PX], bf16, name=f"x{b}", tag="x", bufs=2)
                for c in range(2):
                    nc.sync.dma_start(x_sb[:, c * PX:(c + 1) * PX],
                                      xs[b, c * 128:(c + 1) * 128, :, :])

                # ---- phase 1: xp[m, px] = sum_c w_in[m,c] x[c,px]
                for g in range(NG):
                    p1 = ps.tile([16, 512], f32, name=f"p1_{b}_{g}", tag="p1", bufs=2)
                    for c in range(2):
                        nc.tensor.matmul(
                            p1[:, :], wi_sb[:, c * 16:(c + 1) * 16],
                            x_sb[:, c * PX + g * 512:c * PX + (g + 1) * 512],
                            start=(c == 0), stop=(c == 1))
                    d1 = wk.tile([16, 512], f32, name=f"xps{b}_{g}", tag="xps", bufs=6)
                    if g % 2 == 0:
                        nc.scalar.copy(d1[:, :], p1[:, :])
                    else:
                        nc.vector.tensor_copy(d1[:, :], p1[:, :])
                    nc.sync.dma_start(fwd_in[b][:, g, :], d1[:, :])

                nc.gpsimd.collective_compute(
                    "AllToAll", mybir.AluOpType.bypass, replica_groups=RG,
                    ins=[fwd_in[b].opt()], outs=[fwd_out[b].opt()])

                # ---- FFT chains for this core's 2 images of batch b
                for k in range(2):
                    z = wk.tile([128, 512], f32, name=f"z{b}_{k}", tag="z", bufs=2)
                    for c in range(2):
                        nc.sync.dma_start(z[:, c * 256:(c + 1) * 256],
                                          fwd_out[b][4 * c:4 * c + 4, k, :, :])
                    # s1: A1 = X^T F
                    a1sb = []
                    for h in range(2):
                        q = ps.tile([128, 512], f32, name=f"s1_{b}_{k}_{h}",
                                    tag="fft", bufs=4)
                        for c in range(2):
                            nc.tensor.matmul(
                                q[:, :], z[:, c * 256 + h * 128:c * 256 + h * 128 + 128],
                                c0_sb[:, c * 512:(c + 1) * 512],
                                start=(c == 0), stop=(c == 1))
                        t = wk.tile([128, 512], f32, name=f"a1_{b}_{k}_{h}",
                                    tag="a1", bufs=4)
                        nc.scalar.copy(t[:, :], q[:, :])
                        a1sb.append(t)
                    # s2: S = A1^T F ; gain multiply on drain
                    psb = []
                    for h in range(2):
                        q = ps.tile([128, 512], f32, name=f"s2_{b}_{k}_{h}",
                                    tag="fft", bufs=4)
                        for c in range(2):
                            nc.tensor.matmul(
                                q[:, :], a1sb[c][:, h * 128:h * 128 + 128],
                                c0_sb[:, c * 512:(c + 1) * 512],
                                start=(c == 0), stop=False)
                            nc.tensor.matmul(
                                q[:, :], a1sb[c][:, 256 + h * 128:256 + h * 128 + 128],
                                c3_sb[:, c * 512:(c + 1) * 512],
                                start=False, stop=(c == 1))
                        t = wk.tile([128, 512], f32, name=f"pp_{b}_{k}_{h}",
                                    tag="pp", bufs=4)
                        gsl = gm_sb[:, k * 512 + h * 256:k * 512 + (h + 1) * 256]
                        nc.vector.tensor_mul(t[:, 0:256], q[:, 0:256], gsl)
                        nc.vector.tensor_mul(t[:, 256:512], q[:, 256:512], gsl)
                        psb.append(t)
                    # s3: A3 = P^T conj(F)
                    a3sb = []
                    for h in range(2):
                        q = ps.tile([128, 512], f32, name=f"s3_{b}_{k}_{h}",
                                    tag="fft", bufs=4)
                        for c in range(2):
                            nc.tensor.matmul(
                                q[:, :], psb[c][:, h * 128:h * 128 + 128],
                                c1_sb[:, c * 512:(c + 1) * 512],
                                start=(c == 0), stop=False)
                            nc.tensor.matmul(
                                q[:, :], psb[c][:, 256 + h * 128:256 + h * 128 + 128],
                                c2_sb[:, c * 512:(c + 1) * 512],
                                start=False, stop=(c == 1))
                        t = wk.tile([128, 512], f32, name=f"a3_{b}_{k}_{h}",
                                    tag="a3", bufs=4)
                        nc.scalar.copy(t[:, :], q[:, :])
                        a3sb.append(t)
                    # s4: E = A3r^T Fr + A3i^T Fi  (real part only)
                    xed = wk.tile([128, 512], f32, name=f"xed{b}_{k}", tag="xed", bufs=4)
                    for h in range(2):
                        q = ps.tile([128, 512], f32, name=f"s4_{b}_{k}_{h}",
                                    tag="fft", bufs=4)
                        for c in range(2):
                            nc.tensor.matmul(
                                q[:, 0:256], a3sb[c][:, h * 128:h * 128 + 128],
                                c0_sb[:, c * 512:c * 512 + 256],
                                start=(c == 0), stop=False)
                            nc.tensor.matmul(
                                q[:, 0:256], a3sb[c][:, 256 + h * 128:256 + h * 128 + 128],
                                c0_sb[:, c * 512 + 256:(c + 1) * 512],
                                start=False, stop=(c == 1))
                        nc.vector.tensor_copy(xed[:, h * 256:(h + 1) * 256], q[:, 0:256])
                    for c in range(2):
                        nc.sync.dma_start(bwd_in[b][4 * c:4 * c + 4, k, :, :],
                                          xed[:, c * 256:(c + 1) * 256])

                nc.gpsimd.collective_compute(
                    "AllToAll", mybir.AluOpType.bypass, replica_groups=RG,
                    ins=[bwd_in[b].opt()], outs=[bwd_out[b].opt()])

                # ---- phase 4: out[c, px] = x[c, px] + sum_m w_out[c,m] xe[m, px]
                xeT = []
                for ti in range(2):
                    xt = wk.tile([128, 512], f32, name=f"xe{b}_{ti}", tag="xe", bufs=4)
                    nc.sync.dma_start(xt[:, :], bwd_out[b][:, ti * 8:(ti + 1) * 8, :])
                    xeT.append(xt)
                for g in range(NG):
                    xt = xeT[g // 8]
                    gl = g % 8
                    for h in range(2):
                        p4 = ps.tile([128, 512], f32, name=f"p4_{b}_{g}_{h}",
                                     tag="p4", bufs=2)
                        nc.tensor.matmul(
                            p4[:, :], wo_sb[:, gl * 256 + h * 128:gl * 256 + h * 128 + 128],
                            xt[:, :], start=True, stop=True)
                        osb = wk.tile([128, 512], f32, name=f"osb{b}_{g}_{h}",
                                      tag="osb", bufs=6)
                        nc.vector.tensor_add(
                            osb[:, :], p4[:, :],
                            x_sb[:, h * PX + g * 512:h * PX + (g + 1) * 512])
                        nc.sync.dma_start(
                            out[b, h * 128:(h + 1) * 128, 2 * g:2 * g + 2, :],
                            osb[:, :])

    nc.compile()
    return nc


# -------------------------------------------------------------- host driver
def build_in_maps(x, w_in, w_out, weights):
    g = _gain_masks(np.asarray(weights, np.float32))           # [16, H, W]
    consts = _dft_consts()

    wi = np.zeros((128, 32), np.float32)
    for c in range(2):
        # wi[p, c*16+m] = w_in[m, c*128+p]
        wi[:, c * 16:(c + 1) * 16] = np.asarray(w_in, np.float32)[:, c * 128:(c + 1) * 128].T
    wi = wi.astype(ml_dtypes.bfloat16)

    wo_blk = np.zeros((128, 2048), np.float32)
    w_out = np.asarray(w_out, np.float32)
    for m in range(16):
        for gl in range(8):
            wo_blk[m * 8 + gl, gl * 256:(gl + 1) * 256] = w_out[:, m]

    xbf = np.ascontiguousarray(np.asarray(x, np.float32)).astype(ml_dtypes.bfloat16)

    in_maps = []
    for i in range(NCORES):
        gmc = np.zeros((128, 1024), np.float32)
        for k in range(2):
            gmk = g[2 * i + k]                                  # [256, 256]
            for h in range(2):
                gmc[:, k * 512 + h * 256:k * 512 + (h + 1) * 256] = \
                    gmk[h * 128:(h + 1) * 128, :]
        in_maps.append({
            "xs": np.ascontiguousarray(xbf[:, :, HSH * i:HSH * (i + 1), :]),
            "c0": consts["c0"], "c1": consts["c1"],
            "c2": consts["c2"], "c3": consts["c3"],
            "gm": gmc, "wi": wi, "wo": wo_blk,
        })
    return in_maps


def kernel(x, w_in, w_out, weights):
    import os
    from concourse.bass_utils import run_bass_kernel_spmd

    if "nc" not in _CACHE:
        _CACHE["nc"] = build_graph()
    nc = _CACHE["nc"]

    in_maps = build_in_maps(x, w_in, w_out, weights)
    trace = bool(int(os.environ.get("KERNEL_TRACE", "0")))
    res = run_bass_kernel_spmd(nc, in_maps, core_ids=list(range(NCORES)),
                               trace=trace)
    if trace and res.exec_time_ns is not None:
        print(f"HW exec time: {res.exec_time_ns} ns")
        _CACHE["exec_time_ns"] = res.exec_time_ns

    out = np.concatenate([res.results[i]["out"] for i in range(NCORES)], axis=2)
    return np.ascontiguousarray(out.astype(np.float32))
